# revision 2
# baseline (speedup 1.0000x reference)
"""Trainium2 Bass kernel for causal top-K (K=8) similarity message passing.

Math per batch b (reference):
  gate  = sigmoid(x @ w_gate + b_gate)                      (B,T)
  S     = x @ x^T, causal-masked to NEG=-1e30               (B,T,T)
  top-8 per row -> selected neighbour set, count=min(i+1,8)
  msg   = mean of selected x rows
  blend = mix*x + (1-mix)*msg
  out   = gate * gelu(blend*gain + bias) * (softplus(log_scale)+0.01)

Sharding: 8 cores = 4 batches x 2 query-parity shards. Core c handles
batch b=c>>1, parity p=c&1, processing query tiles Qg = 2t+p (t=0..T/256-1)
of 128 rows each. Every core runs a uniform program; all parity
dependence is carried in per-core input data (masks / precomputed tables).

Numerics: scores need enough precision that the top-8 *selection* matches
fp32 (rank-8/9 gaps are O(1) while |score| is O(100); bf16 or f32r alone
flips a few % of rows and blows the 2e-2 budget). x^T is therefore split
host-side into bf16 hi+lo halves and S accumulated as
hi*hi + hi*lo + lo*hi in PSUM -- fp32-grade scores at 3 bf16 matmul
passes (vs 4 passes for native fp32 PE). The aggregation path
(sel^T @ x*gain) is plain bf16: msg is a mean of <=8 unit-scale rows, so
bf16 rounding lands ~1e-3 relative, far inside the 2e-2 budget.

Per query tile t (Lc = 2t+2 key chunks of 128; one wasted fully-masked
chunk for p=0 so both parities run the identical program):
  scores  = sum of 3 split matmuls (PE, bf16) -> PSUM -> SBUF + causal mask
  v8      = max8(scores), tau = v8[:,7]      (DVE top-8 instruction)
  sel     = scores >= tau (0/1), fixed up for tile 0; diagonal gets
            mix*count/(1-mix) added so the blend's mix*x term rides the
            aggregation matmul
  msg     = sel^T-transposed chunks @ (x*gain bf16) accumulated in PSUM,
            plus a rank-1 matmul adding bias*count/(1-mix)
  z       = msg * (1-mix)/count   (per-row scale during PSUM->SBUF copy)
  out     = sigmoid(gate_lin)*scale * gelu(z)

The whole per-core computation sits inside a tc.For_i whose trip count is
read at runtime from the tiny `iters` input tensor, so a single compiled
program (~1 min compile) serves both correctness (iters=1) and marginal
HW timing (iters=R vs 1) without recompiling.
"""

import os
import sys

for _p in ("/opt/trn_rl_repo", os.path.expanduser("~/.axon_site/_ro/trn_rl_repo")):
    if os.path.isdir(_p) and _p not in sys.path:
        sys.path.insert(0, _p)
        break

import numpy as np
import ml_dtypes

import concourse.bacc as bacc
import concourse.mybir as mybir
from concourse import masks
from concourse.tile import TileContext
from concourse.bass_utils import run_bass_kernel_spmd

F32 = mybir.dt.float32
BF16 = mybir.dt.bfloat16
I32 = mybir.dt.int32
AF = mybir.ActivationFunctionType
ALU = mybir.AluOpType
NEG = np.float32(-1e30)
NPBF = ml_dtypes.bfloat16

D = 1024
DC = 8  # D // 128
N_CORES = 8

_prog_cache = {}


def build_program(T, stage=5):
    """Build + compile the uniform per-core program for sequence length T."""
    key = (T, stage)
    if key in _prog_cache:
        return _prog_cache[key]

    NQT = T // 256  # query tiles per core
    nc = bacc.Bacc(trn_type="TRN2", target_bir_lowering=False, debug=False,
                   num_devices=N_CORES, dynamic_dma_scratch_size=512)

    xh_in = nc.dram_tensor("xh", [128, DC, T], BF16, kind="ExternalInput").ap()
    xl_in = nc.dram_tensor("xl", [128, DC, T], BF16, kind="ExternalInput").ap()
    qh_in = nc.dram_tensor("qh", [NQT, 128, DC, 128], BF16,
                           kind="ExternalInput").ap()
    ql_in = nc.dram_tensor("ql", [NQT, 128, DC, 128], BF16,
                           kind="ExternalInput").ap()
    xg_in = nc.dram_tensor("xg", [T, D], BF16, kind="ExternalInput").ap()
    qmask_in = nc.dram_tensor("qmask", [128, 256], F32, kind="ExternalInput").ap()
    smask_in = nc.dram_tensor("smask", [128, 256], F32, kind="ExternalInput").ap()
    dmask_in = nc.dram_tensor("dmask", [128, 256], F32, kind="ExternalInput").ap()
    dmask0_in = nc.dram_tensor("dmask0", [128, 256], F32, kind="ExternalInput").ap()
    recip_in = nc.dram_tensor("recipc", [128, NQT], F32, kind="ExternalInput").ap()
    eta_in = nc.dram_tensor("eta", [1, NQT, 128], BF16, kind="ExternalInput").ap()
    biasr_in = nc.dram_tensor("biasr", [1, D], BF16, kind="ExternalInput").ap()
    wg_in = nc.dram_tensor("wg", [128, DC], BF16, kind="ExternalInput").ap()
    sc_in = nc.dram_tensor("sc", [128, 2], F32, kind="ExternalInput").ap()
    it_h = nc.dram_tensor("iters", [1, 1], I32, kind="ExternalInput")
    y_out = nc.dram_tensor("y", [NQT, 128, D], F32, kind="ExternalOutput").ap()

    from contextlib import ExitStack

    with TileContext(nc) as tc, ExitStack() as ctx:
        cpool = ctx.enter_context(tc.tile_pool(name="consts", bufs=1))
        xTp = ctx.enter_context(tc.tile_pool(name="xTp", bufs=1))
        Sp = ctx.enter_context(tc.tile_pool(name="Sp", bufs=2))
        xkp = ctx.enter_context(tc.tile_pool(name="xkp", bufs=3))
        xqp = ctx.enter_context(tc.tile_pool(name="xqp", bufs=2))
        stp = ctx.enter_context(tc.tile_pool(name="stp", bufs=3))
        msgp = ctx.enter_context(tc.tile_pool(name="msgp", bufs=2))
        smallp = ctx.enter_context(tc.tile_pool(name="smallp", bufs=2))
        ps_s = ctx.enter_context(tc.tile_pool(name="ps_s", bufs=2, space="PSUM"))
        ps_t = ctx.enter_context(tc.tile_pool(name="ps_t", bufs=1, space="PSUM"))
        ps_m = ctx.enter_context(tc.tile_pool(name="ps_m", bufs=2, space="PSUM"))
        ps_g = ctx.enter_context(tc.tile_pool(name="ps_g", bufs=1, space="PSUM"))

        qmask = cpool.tile([128, 256], F32)
        nc.sync.dma_start(out=qmask[:], in_=qmask_in[:])
        smask = cpool.tile([128, 256], F32)
        nc.sync.dma_start(out=smask[:], in_=smask_in[:])
        dmask = cpool.tile([128, 256], F32)
        nc.sync.dma_start(out=dmask[:], in_=dmask_in[:])
        dmask0 = cpool.tile([128, 256], F32)
        nc.sync.dma_start(out=dmask0[:], in_=dmask0_in[:])
        recip = cpool.tile([128, NQT], F32)
        nc.sync.dma_start(out=recip[:], in_=recip_in[:])
        eta = cpool.tile([1, NQT, 128], BF16)
        nc.sync.dma_start(out=eta[:], in_=eta_in[:])
        biasr = cpool.tile([1, D], BF16)
        nc.sync.dma_start(out=biasr[:], in_=biasr_in[:])
        wg = cpool.tile([128, DC], BF16)
        nc.sync.dma_start(out=wg[:], in_=wg_in[:])
        sc = cpool.tile([128, 2], F32)
        nc.sync.dma_start(out=sc[:], in_=sc_in[:])
        ident32 = cpool.tile([128, 128], F32)
        masks.make_identity(nc, ident32[:])

        itreg = nc.alloc_registers("iters_reg", mybir.ALL_ENGINES)
        nc.regs_load(itreg, it_h[0:1, 0:1])
        iters_sv = nc.snap(itreg, donate=True, min_val=0, max_val=1 << 20)

        with tc.For_i(0, iters_sv, name="reps") as _rep:
            # resident bf16 x^T halves, re-DMA'd each rep (honest HBM accounting)
            xh = xTp.tile([128, DC, T], BF16, tag="xh", name="xh")
            nc.sync.dma_start(out=xh[:], in_=xh_in[:])
            xl = xTp.tile([128, DC, T], BF16, tag="xl", name="xl")
            nc.sync.dma_start(out=xl[:], in_=xl_in[:])

            for t in range(NQT):
                Lc = 2 * t + 2
                Lk = Lc * 128
                qht = xqp.tile([128, DC, 128], BF16, tag="qh", name="qh")
                nc.sync.dma_start(out=qht[:], in_=qh_in[t])
                qlt = xqp.tile([128, DC, 128], BF16, tag="ql", name="ql")
                nc.sync.dma_start(out=qlt[:], in_=ql_in[t])

                # ---- scores: hi*hi + hi*lo + lo*hi ----
                S = Sp.tile([128, T], F32)
                nblk = (Lk + 511) // 512
                for blk in range(nblk):
                    w = min(512, Lk - blk * 512)
                    lo = blk * 512
                    ps = ps_s.tile([128, 512], F32)
                    n3 = 3 * DC
                    i = 0
                    for qt, xt in ((qht, xh), (qht, xl), (qlt, xh)):
                        for dc in range(DC):
                            nc.tensor.matmul(ps[:, :w], qt[:, dc],
                                             xt[:, dc, lo:lo + w],
                                             start=(i == 0), stop=(i == n3 - 1))
                            i += 1
                    plain_w = w if blk < nblk - 1 else w - 256
                    if plain_w > 0:
                        nc.scalar.copy(S[:, lo:lo + plain_w], ps[:, :plain_w])
                    if blk == nblk - 1:
                        nc.vector.tensor_add(S[:, Lk - 256:Lk],
                                             ps[:, w - 256:w], qmask[:])

                if stage <= 2:
                    dbg2 = msgp.tile([128, D], F32, name="dbg2")
                    nc.vector.tensor_copy(dbg2[:], S[:, 0:D])
                    nc.sync.dma_start(out=y_out[t], in_=dbg2[:])
                    continue

                # ---- top-8 threshold -> selection weights in-place ----
                v8 = smallp.tile([128, 8], F32, tag="v8", name="v8")
                nc.vector.max(out=v8[:], in_=S[:, :Lk])
                nc.vector.tensor_scalar(S[:, :Lk], S[:, :Lk], v8[:, 7:8], None,
                                        op0=ALU.is_ge)
                if t == 0:
                    nc.vector.tensor_mul(S[:, :256], S[:, :256], smask[:])
                dm = dmask0 if t == 0 else dmask
                nc.vector.tensor_add(S[:, Lk - 256:Lk], S[:, Lk - 256:Lk], dm[:])

                if stage <= 3:
                    dbg3 = msgp.tile([128, D], F32, name="dbg3")
                    nc.vector.tensor_copy(dbg3[:], S[:, 0:D])
                    nc.sync.dma_start(out=y_out[t], in_=dbg3[:])
                    continue

                # ---- gate ----
                pg = ps_g.tile([128, 1], F32)
                for dc in range(DC):
                    nc.tensor.matmul(pg[:], qht[:, dc], wg[:, dc:dc + 1],
                                     start=(dc == 0), stop=False)
                for dc in range(DC):
                    nc.tensor.matmul(pg[:], qlt[:, dc], wg[:, dc:dc + 1],
                                     start=False, stop=(dc == DC - 1))
                gate = smallp.tile([128, 1], F32, tag="gate", name="gate")
                nc.scalar.activation(gate[:], pg[:], AF.Sigmoid,
                                     bias=sc[:, 0:1], scale=1.0)
                nc.vector.tensor_mul(gate[:], gate[:], sc[:, 1:2])

                # ---- aggregation: msg psum = sel^T @ (x*gain) + eta*bias ----
                pm = ps_m.tile([128, D], F32)
                for h in (0, 1):
                    nc.tensor.matmul(pm[:, h * 512:(h + 1) * 512],
                                     eta[0:1, t], biasr[0:1, h * 512:(h + 1) * 512],
                                     start=True, stop=False)
                for c in range(Lc):
                    pt = ps_t.tile([128, 128], F32, tag="pt", name="pts")
                    nc.tensor.transpose(pt[:], S[:, c * 128:(c + 1) * 128],
                                        ident32[:])
                    sT = stp.tile([128, 128], BF16)
                    nc.scalar.copy(sT[:], pt[:])
                    xk = xkp.tile([128, D], BF16, tag="xk", name="xk")
                    nc.sync.dma_start(out=xk[:], in_=xg_in[c * 128:(c + 1) * 128, :])
                    for h in (0, 1):
                        nc.tensor.matmul(pm[:, h * 512:(h + 1) * 512], sT[:],
                                         xk[:, h * 512:(h + 1) * 512],
                                         start=False, stop=(c == Lc - 1))

                if stage <= 4:
                    dbg4 = msgp.tile([128, D], F32, name="dbg4")
                    nc.scalar.copy(dbg4[:], pm[:])
                    nc.sync.dma_start(out=y_out[t], in_=dbg4[:])
                    continue

                # ---- tail: z = pm*recip; out = gate * gelu(z) ----
                msg = msgp.tile([128, D], F32)
                nc.scalar.activation(msg[:], pm[:], AF.Copy,
                                     scale=recip[:, t:t + 1])
                nc.scalar.activation(msg[:], msg[:], AF.Gelu)
                nc.vector.tensor_scalar(msg[:], msg[:], gate[:, 0:1], None,
                                        op0=ALU.mult)
                nc.sync.dma_start(out=y_out[t], in_=msg[:])

    nc.compile()
    _prog_cache[key] = nc
    return nc


def host_inputs(xb, p, mix, scale, b_gate, w_gate, gain, bias, T, iters):
    """Per-core input arrays for batch slice xb (T,D) and parity p."""
    NQT = T // 256
    f32 = np.float32
    xb = np.ascontiguousarray(xb, f32)

    xh2 = xb.astype(NPBF)                       # (T,D) hi
    xl2 = (xb - xh2.astype(f32)).astype(NPBF)   # (T,D) lo
    # x^T halves: [128=dp, DC, T]
    xh = np.ascontiguousarray(xh2.reshape(T, DC, 128).transpose(2, 1, 0))
    xl = np.ascontiguousarray(xl2.reshape(T, DC, 128).transpose(2, 1, 0))
    # query-tile slices of the transposed halves: [NQT, 128=dp, DC, 128=q]
    rows_h = xh2.reshape(T // 128, 128, D)[p::2]
    rows_l = xl2.reshape(T // 128, 128, D)[p::2]
    qh = np.ascontiguousarray(
        rows_h.reshape(NQT, 128, DC, 128).transpose(0, 3, 2, 1))
    ql = np.ascontiguousarray(
        rows_l.reshape(NQT, 128, DC, 128).transpose(0, 3, 2, 1))

    r = np.arange(128)
    tri_add = np.where(r[None, :] <= r[:, None], f32(0), NEG).astype(f32)
    tri01 = (r[None, :] <= r[:, None]).astype(f32)
    qmask = np.zeros((128, 256), f32)
    smask = np.zeros((128, 256), f32)
    if p == 0:
        qmask[:, :128] = tri_add
        qmask[:, 128:] = NEG
        smask[:, :128] = tri01
    else:
        qmask[:, 128:] = tri_add
        smask[:, :128] = 1.0
        smask[:, 128:] = tri01

    # counts: count(t, q) = min((2t+p)*128 + q + 1, 8)
    g_row = (2 * np.arange(NQT)[:, None] + p) * 128 + r[None, :]  # (NQT,128)
    counts = np.minimum(g_row + 1, 8).astype(f32)

    dmask = np.zeros((128, 256), f32)
    dmask0 = np.zeros((128, 256), f32)
    half = 0 if p == 0 else 128
    mixfac_n = mix * 8.0 / (1.0 - mix)
    mixfac_0 = mix * counts[0] / (1.0 - mix)
    dmask[r, half + r] = mixfac_n
    dmask0[r, half + r] = mixfac_0

    recipc = np.ascontiguousarray(((1.0 - mix) / counts).T)     # (128, NQT)
    eta = np.ascontiguousarray((counts / (1.0 - mix))[None])    # (1, NQT, 128)

    wg = np.ascontiguousarray(np.asarray(w_gate, f32).reshape(DC, 128).T)
    sc_arr = np.zeros((128, 2), f32)
    sc_arr[:, 0] = b_gate
    sc_arr[:, 1] = scale
    return {
        "xh": xh, "xl": xl, "qh": qh, "ql": ql,
        "xg": np.ascontiguousarray(
            (xb * np.asarray(gain, f32)[None, :]).astype(NPBF)),
        "qmask": qmask, "smask": smask, "dmask": dmask, "dmask0": dmask0,
        "recipc": recipc.astype(f32), "eta": eta.astype(NPBF),
        "biasr": np.ascontiguousarray(np.asarray(bias, f32)[None, :]).astype(NPBF),
        "wg": wg.astype(NPBF),
        "sc": sc_arr,
        "iters": np.array([[iters]], np.int32),
    }


def run_cores(x, w_gate, b_gate, gain, bias, log_mix, log_scale,
              iters=1, bench=False, stage=5):
    """Run the SPMD program over all 8 cores; returns (B,T,D) output."""
    x = np.asarray(x, np.float32)
    B, T, _ = x.shape
    mix = float(1.0 / (1.0 + np.exp(-np.float64(log_mix))))
    scale = float(np.logaddexp(0.0, np.float64(log_scale)) + 0.01)
    b_gate_f = float(np.asarray(b_gate, np.float64))

    nc = build_program(T, stage=stage)
    in_maps = []
    for core in range(N_CORES):
        b, p = core >> 1, core & 1
        in_maps.append(host_inputs(x[b % B], p, mix, scale, b_gate_f,
                                   w_gate, gain, bias, T, iters))
    res = run_bass_kernel_spmd(nc, in_maps, list(range(N_CORES)))
    if bench:
        return None
    out = np.empty((B, T, D), np.float32)
    for core in range(N_CORES):
        b, p = core >> 1, core & 1
        if b >= B:
            continue
        out[b].reshape(T // 128, 128, D)[p::2] = res.results[core]["y"]
    return out


def kernel(x, w_gate, b_gate, gain, bias, log_mix, log_scale, K):
    assert int(K) == 8, "kernel is specialized for K=8"
    return run_cores(x, w_gate, b_gate, gain, bias, log_mix, log_scale)
